# revision 4
# baseline (speedup 1.0000x reference)
"""Multi-head attention (16 heads, S=2048, d_model=1024, d_head=64) on 8 TRN2
NeuronCores, tensor-parallel over heads (2 heads per core).

v3, restructured around the TimelineSim cost model (matmul cost = output
free-size rows; ACT/DVE/Pool cost = max-operand free size x engine cycle):

  * AV matmuls transposed: out[sq=128, dv+1=65], exp tile stationary -> 65
    rows per accumulation step; softmax normalize is a per-partition
    tensor_scalar against the ride-along denominator column.
  * everything 2-byte through the matmuls; exp computes exp(z/8 - 4) so
    scores fit fp16, bias cancels in the normalize.
  * engine rebalance: 16 of 64 exp tiles run as a Schraudolph bit-trick
    split across DVE (tmp = z*A+B, the only PSUM-capable stage) and Pool
    (int16 max+convert, SBUF-only) -- Pool is otherwise idle and ACT drops
    from ~72us busy to ~55us; copies split ~28 DVE / ~4 ACT.
  * two HWDGE queues: SP carries wk + K/V chunks + out DMAs, ACT carries
    wq/wv/wot + Q chunks + hsq DMA-transposes. First Q/K chunks are 256
    cols and the first z pair-tile exps split by sq-half, pulling the
    first exp from ~9.4us to ~6.5us.
  * hsq [sq,hd] -> headst [hd,sq] via DMA-engine XBAR transpose (14ns per
    16x128 tile) instead of PE transpose for all but the first 3 units
    (those fire while input DMA still owns the device) -- saves 2k PE rows
    and the tr PSUM tile.

Sq splits into groups [512,512,512,128,128,128,128]; packed 128-wide tail
groups keep exps dense and make each finish chain a single sq-tile.
PSUM: 2x[128,1024] z + 2x[128,130] paired-head AV accumulators +
2x[128,512] proj/outproj = exactly 8 banks.
"""

import os

import numpy as np

import concourse.bass as bass
import concourse.tile as tile
from concourse import bacc, mybir
from concourse.bass_utils import run_bass_kernel_spmd

HEADS, D_K, D_V, D_X, D_M, S = 16, 64, 64, 1024, 1024, 2048
NCORES = 8
HPC = HEADS // NCORES          # heads per core
HD = HPC * D_K                 # 128: stacked head dim per core
SQW = 512                      # sq group width
NSQ = S // SQW                 # 4 groups
SKW = 128                      # sk chunk width (partition dim)
NSK = S // SKW                 # 16
NXC = D_X // 128               # 8 contraction chunks for projections
NJJ = NSK // 2                 # 8 skc-pairs (one [128,1024] z tile each)
GROUPS = [(0, 512), (512, 512), (1024, 512),
          (1536, 128), (1664, 128), (1792, 128), (1920, 128)]

F32 = mybir.dt.float32
F16 = mybir.dt.float16
I16 = mybir.dt.int16
EXP = mybir.ActivationFunctionType.Exp
# Schraudolph fp16 exp constants: exp(z/8-4) = 2^(z*0.125*log2e - 4*log2e),
# int16(1024*(t + 15 - C)) bitcast as fp16 ~ 2^t; C tuned for min rms (1.9%)
LOG2E = 1.4426950408889634
SCH_A = 0.125 * LOG2E * 1024.0
SCH_B = 1024.0 * (15.0 - 4.0 * LOG2E - 0.0545)

# z tiles whose exp runs on DVE (stage1) + Pool (stage2) instead of ACT
SPLIT_TILES = {(0, 3), (0, 5), (1, 1), (1, 3), (1, 5), (2, 1), (2, 3), (2, 5)}

LAST_EXEC_NS = None
_NC_CACHE = None


def _emit(tc, nc, aps):
    from contextlib import ExitStack

    qt, kt, vt, wq, wk, wv, wot, ident, out = (
        aps["qt"], aps["kt"], aps["vt"], aps["wq"], aps["wk"], aps["wv"],
        aps["wot"], aps["ident"], aps["out"],
    )

    with ExitStack() as ctx:
        wpool = ctx.enter_context(tc.tile_pool(name="weights", bufs=1))
        proj = ctx.enter_context(tc.tile_pool(name="proj", bufs=1))
        inp = ctx.enter_context(tc.tile_pool(name="inp", bufs=5))
        etp = ctx.enter_context(tc.tile_pool(name="et", bufs=46))
        hsqp = ctx.enter_context(tc.tile_pool(name="hsq", bufs=10))
        outp = ctx.enter_context(tc.tile_pool(name="outs", bufs=6))
        smalls = ctx.enter_context(tc.tile_pool(name="smalls", bufs=4))
        ps_z = ctx.enter_context(tc.tile_pool(name="ps_z", bufs=2, space="PSUM"))
        ps_av = ctx.enter_context(tc.tile_pool(name="ps_av", bufs=2, space="PSUM"))
        ps_pr = ctx.enter_context(tc.tile_pool(name="ps_pr", bufs=2, space="PSUM"))

        # ---- persistent SBUF tensors ----
        wq_sb = wpool.tile([128, D_X], F16, tag="wq")     # (xc p) stacked chunks
        wk_sb = wpool.tile([128, D_X], F16, tag="wk")
        wv_sb = wpool.tile([128, D_X], F16, tag="wv")
        wot_sb = wpool.tile([HD, D_M], F16, tag="wot")
        ident_sb = wpool.tile([128, 128], F16, tag="ident")
        qpt_sb = proj.tile([HD, S], F16, tag="qpt")
        kpt_sb = proj.tile([HD, S], F16, tag="kpt")
        # VpAug: per (h, skc) a (128 sk, 65) block: cols 0-63 = Vp, col 64 = 1
        vpa_sb = proj.tile([128, HPC * NSK * 65], F16, tag="vpa")
        headst_sb = proj.tile([HD, S], F16, tag="headst")

        def load_w(w_dram, w_sb, q=None):
            (q or nc.sync).dma_start(w_sb[:], w_dram)

        def load_chunk(tt_dram, c, name, lo=0, w=SQW, tag="inp", q=None):
            """One DMA: all 8 xc strips of cols [c*512+lo, +w) -> (128, 8, w)."""
            t = inp.tile([128, NXC, w], F16, tag=tag, name=name,
                         bufs=4 if tag == "inp0" else None)
            (q or nc.sync).dma_start(
                t[:],
                tt_dram.rearrange("(xc p) s -> p xc s", p=128)[
                    :, :, c * SQW + lo:c * SQW + lo + w
                ],
            )
            return t

        def project(t, w_sb, dst_sb, c, name, lo=0, w=SQW, copy_eng=None,
                    t_lo=0):
            """dst_sb[:, c*512+lo : +w] = W.T @ X.T chunk cols (fp16)."""
            ps = ps_pr.tile([128, w], F32, tag="pr", name=name,
                            padded_shape=[128, SQW])
            for xc in range(NXC):
                nc.tensor.matmul(
                    ps[:],
                    w_sb[:, xc * 128:(xc + 1) * 128],
                    t[:, xc, lo - t_lo:lo - t_lo + w],
                    start=(xc == 0),
                    stop=(xc == NXC - 1),
                )
            dst = dst_sb[:, c * SQW + lo:c * SQW + lo + w]
            if copy_eng == "split":
                nc.vector.tensor_copy(dst[:, 0:w // 2], ps[:, 0:w // 2])
                nc.scalar.copy(dst[:, w // 2:w], ps[:, w // 2:w])
            elif copy_eng == "scalar":
                nc.scalar.copy(dst, ps[:])
            else:
                nc.vector.tensor_copy(dst, ps[:])

        def project_v(t, c):
            """VpAug sk-chunks for 512-chunk c: Vp = VT_chunk.T @ Wv directly
            in (sk, hd) layout."""
            for j in range(SQW // SKW):
                skc = c * (SQW // SKW) + j
                ps = ps_pr.tile([128, HD], F32, tag="pr", name=f"vp_{skc}",
                                padded_shape=[128, SQW])
                for xc in range(NXC):
                    nc.tensor.matmul(
                        ps[:],
                        t[:, xc, j * SKW:(j + 1) * SKW],
                        wv_sb[:, xc * 128:(xc + 1) * 128],
                        start=(xc == 0),
                        stop=(xc == NXC - 1),
                    )
                # both heads in one strided copy: vpa block stride is 16*65
                nc.vector.tensor_copy(
                    vpa_sb[:].rearrange("p (h f) -> p h f", h=HPC)[
                        :, :, skc * 65:skc * 65 + 64],
                    ps[:].rearrange("p (h f) -> p h f", h=HPC),
                )

        ets = {}  # (h, g, jj) -> ET tile awaiting its AV matmuls
        zhead = {}  # (h, jj) -> (z_ps, et) for the sq-split head tiles

        def z_head_part(jj, part):
            """First z pair-tiles, computed in sq-256 halves so the exp
            stream starts as soon as the first 256 cols of Q land."""
            lo = part * 256
            for h in range(HPC):
                if part == 0:
                    z_ps = ps_z.tile([128, 2 * SQW], F32, tag="z",
                                     name=f"z_{h}_0_{jj}")
                    et = etp.tile([128, 2 * SQW], F16, tag="et",
                                  name=f"et_{h}_0_{jj}")
                    zhead[(h, jj)] = (z_ps, et)
                    ets[(h, 0, jj)] = et
                z_ps, et = zhead[(h, jj)]
                for half in range(2):
                    skc = 2 * jj + half
                    nc.tensor.matmul(
                        z_ps[:, half * SQW + lo:half * SQW + lo + 256],
                        kpt_sb[h * 64:(h + 1) * 64, skc * SKW:(skc + 1) * SKW],
                        qpt_sb[h * 64:(h + 1) * 64, lo:lo + 256],
                        start=True,
                        stop=True,
                    )
                zin = z_ps[:].rearrange("p (c w) -> p c w", c=2)[
                    :, :, lo:lo + 256]
                eout = et[:].rearrange("p (c w) -> p c w", c=2)[
                    :, :, lo:lo + 256]
                nc.scalar.activation(eout, zin, EXP, scale=1.0 / 8.0,
                                     bias=bias_sb[:])

        def z_exp(g, jj):
            """Per head: one [128, 2W] z tile (skc pair 2jj,2jj+1) + exp.

            exp(z/8 - 4): the -4 bias keeps the scores in fp16 range and
            cancels in the normalize. Tiles in SPLIT_TILES compute a
            Schraudolph bit-trick exp instead: DVE does tmp = z*A+B (the
            PSUM read), Pool does the int16 max/convert; ~1.9% rms score
            error that largely cancels num/denom.
            """
            lo, w = GROUPS[g]
            split = (g, jj) in SPLIT_TILES
            for h in range(HPC):
                z_ps = ps_z.tile([128, SQW + w], F32, tag="z",
                                 name=f"z_{h}_{g}_{jj}",
                                 padded_shape=[128, 2 * SQW])
                for half in range(2):
                    skc = 2 * jj + half
                    nc.tensor.matmul(
                        z_ps[:, half * SQW:half * SQW + w],
                        kpt_sb[h * 64:(h + 1) * 64, skc * SKW:(skc + 1) * SKW],
                        qpt_sb[h * 64:(h + 1) * 64, lo:lo + w],
                        start=True,
                        stop=True,
                    )
                et = etp.tile([128, SQW + w], F16, tag="et",
                              name=f"et_{h}_{g}_{jj}",
                              padded_shape=[128, 2 * SQW])
                if split:
                    tmp = smalls.tile([128, SQW + w], F32, tag="tmp", bufs=3,
                                      name=f"tmp_{h}_{g}_{jj}",
                                      padded_shape=[128, 2 * SQW])
                    nc.vector.tensor_scalar(
                        tmp[:], z_ps[:], SCH_A, SCH_B,
                        mybir.AluOpType.mult, mybir.AluOpType.add)
                    nc.gpsimd.tensor_scalar_max(
                        et[:].bitcast(I16), tmp[:], 0.0)
                else:
                    nc.scalar.activation(et[:], z_ps[:], EXP,
                                         scale=1.0 / 8.0, bias=bias_sb[:])
                ets[(h, g, jj)] = et

        def z_pack(g, i):
            """128-wide group: one tile packs 8 skc at col k*128."""
            lo, w = GROUPS[g]
            for h in range(HPC):
                z_ps = ps_z.tile([128, 2 * SQW], F32, tag="z",
                                 name=f"zp_{g}_{h}_{i}")
                for k in range(8):
                    skc = 8 * i + k
                    nc.tensor.matmul(
                        z_ps[:, k * 128:(k + 1) * 128],
                        kpt_sb[h * 64:(h + 1) * 64, skc * SKW:(skc + 1) * SKW],
                        qpt_sb[h * 64:(h + 1) * 64, lo:lo + w],
                        start=True,
                        stop=True,
                    )
                et = etp.tile([128, 2 * SQW], F16, tag="et",
                              name=f"etp_{g}_{h}_{i}")
                nc.scalar.activation(et[:], z_ps[:], EXP,
                                     scale=1.0 / 8.0, bias=bias_sb[:])
                ets[(h, g, i)] = et

        # PSUM accumulation groups must be contiguous per bank, so AV runs
        # as per-(h, sq-128-subtile) bursts of 16 back-to-back matmuls.
        _avalt = [0]

        def alloc_acc(name):
            _avalt[0] ^= 1
            return ps_av.tile([128, 130], F32, tag=f"av{_avalt[0]}", bufs=1,
                              name=name)

        def av_run(g, h, m, acc0):
            acc = acc0[:, h * 65:(h + 1) * 65]
            if GROUPS[g][1] == 128:
                for skc in range(NSK):
                    et = ets[(h, g, skc // 8)]
                    vb = (h * NSK + skc) * 65
                    nc.tensor.matmul(
                        acc[:],
                        et[:, (skc % 8) * 128:(skc % 8 + 1) * 128],
                        vpa_sb[:, vb:vb + 65],
                        start=(skc == 0),
                        stop=(skc == NSK - 1),
                    )
                return
            for jj in range(NJJ):
                et = ets[(h, g, jj)]
                for half in range(2):
                    skc = 2 * jj + half
                    vb = (h * NSK + skc) * 65
                    nc.tensor.matmul(
                        acc[:],
                        et[:, half * SQW + m * 128:half * SQW + (m + 1) * 128],
                        vpa_sb[:, vb:vb + 65],
                        start=(skc == 0),
                        stop=(skc == NSK - 1),
                    )

        def unit_front(g, m, tail=False):
            """AV runs + normalize -> hsq in SBUF."""
            t = GROUPS[g][0] // 128 + m
            acc = alloc_acc(f"acc_{t}")
            for h in range(HPC):
                av_run(g, h, m, acc)
            hsq = hsqp.tile([128, HD], F16, tag="hsq", name=f"hsq_{t}")
            rec = smalls.tile([128, 2], F32, tag="rec", name=f"rec_{t}")
            nc.vector.reciprocal(
                rec[:].rearrange("p (h c) -> p h c", c=1),
                acc[:].rearrange("p (h c) -> p h c", c=65)[:, :, 64:65])
            for h in range(HPC):
                dsth = hsq[:, h * 64:(h + 1) * 64]
                if tail and h == 1:
                    nc.scalar.mul(dsth, acc[:, h * 65:h * 65 + 64],
                                  rec[:, h:h + 1])
                else:
                    nc.vector.tensor_scalar_mul(
                        dsth, acc[:, h * 65:h * 65 + 64], rec[:, h:h + 1])
            return hsq

        def unit_back(g, m, hsq, tail=False, pe_tr=False):
            """transpose + outproj + DMA; a z-slot after its front."""
            t = GROUPS[g][0] // 128 + m
            hdst = headst_sb[:, t * 128:(t + 1) * 128]
            if pe_tr:
                tr = ps_pr.tile([128, SQW], F16, tag="pr", name=f"tr_{t}")
                nc.tensor.transpose(tr[:, 0:128], hsq[:], ident_sb[:])
                nc.vector.tensor_copy(hdst, tr[:, 0:128])
            else:
                # XBAR transpose on the (mostly idle) DMA engines, ACT queue
                nc.scalar.dma_start(hdst, hsq[:], transpose=True)
            ot = outp.tile([128, D_M], F16, tag="ot", name=f"ot_{t}")
            for dmc in range(D_M // SQW):
                op = ps_pr.tile([128, SQW], F32, tag="pr", name=f"op_{t}_{dmc}")
                nc.tensor.matmul(
                    op[:],
                    headst_sb[:, t * 128:(t + 1) * 128],
                    wot_sb[:, dmc * SQW:(dmc + 1) * SQW],
                    start=True,
                    stop=True,
                )
                dst = ot[:, dmc * SQW:(dmc + 1) * SQW]
                if (tail and dmc % 2) or (not tail and (2 * t + dmc) % 8 == 3):
                    nc.scalar.copy(dst, op[:])
                else:
                    nc.vector.tensor_copy(dst, op[:])
                if tail:
                    nc.sync.dma_start(
                        out[t * 128:(t + 1) * 128, dmc * SQW:(dmc + 1) * SQW],
                        ot[:, dmc * SQW:(dmc + 1) * SQW])
            if not tail:
                nc.sync.dma_start(out[t * 128:(t + 1) * 128, :], ot[:])

        # ---- gpsimd constants (no DMA needed) ----
        nc.gpsimd.memset(
            vpa_sb[:].rearrange("p (c f) -> p c f", f=65)[:, :, 64:65], 1.0)
        bias_sb = wpool.tile([128, 1], F32, tag="bias")
        nc.gpsimd.memset(bias_sb[:], -4.0)

        # absorb the 1.3us exp table load inside the initial DMA window;
        # emitted before the ACT-queue DMAs so it dispatches immediately
        warm = smalls.tile([128, 1], F32, tag="warm")
        nc.scalar.activation(warm[:], bias_sb[:], EXP, scale=1.0)

        # ---- DMA streams ----
        # SP queue: wk + K/V chunks (+ out DMAs later)
        # ACT queue: wq/wv/wot + Q chunks (+ hsq transposes later)
        tk, tq, tv = {}, {}, {}
        load_w(wk, wk_sb, q=nc.sync)
        load_w(wq, wq_sb, q=nc.scalar)
        q0a = load_chunk(qt, 0, "qc_0a", lo=0, w=256, tag="inp0", q=nc.scalar)
        k0a = load_chunk(kt, 0, "kc_0a", lo=0, w=256, tag="inp0", q=nc.sync)
        q0b = load_chunk(qt, 0, "qc_0b", lo=256, w=256, tag="inp0",
                         q=nc.scalar)
        k0b = load_chunk(kt, 0, "kc_0b", lo=256, w=256, tag="inp0", q=nc.sync)
        tq[1] = load_chunk(qt, 1, "qc_1", q=nc.scalar)
        tk[1] = load_chunk(kt, 1, "kc_1", q=nc.sync)
        nc.sync.dma_start(ident_sb[:], ident)
        load_w(wv, wv_sb, q=nc.scalar)
        tv[0] = load_chunk(vt, 0, "vc_0", q=nc.sync)
        tq[2] = load_chunk(qt, 2, "qc_2", q=nc.scalar)
        tk[2] = load_chunk(kt, 2, "kc_2", q=nc.sync)
        tv[1] = load_chunk(vt, 1, "vc_1", q=nc.sync)
        load_w(wot, wot_sb, q=nc.scalar)
        tk[3] = load_chunk(kt, 3, "kc_3", q=nc.sync)
        tq[3] = load_chunk(qt, 3, "qc_3", q=nc.scalar)
        tv[2] = load_chunk(vt, 2, "vc_2", q=nc.sync)
        tv[3] = load_chunk(vt, 3, "vc_3", q=nc.sync)

        # burn the PE pstate ramp on a few junk matmuls over memset data
        jsrc = wpool.tile([128, 128], F16, tag="jsrc")
        nc.gpsimd.memset(jsrc[:], 0.0)
        junk = ps_pr.tile([128, SQW], F32, tag="pr", name="junk")
        for _ in range(20):
            nc.tensor.matmul(junk[:, 0:128], jsrc[:], jsrc[:],
                             start=True, stop=True)

        # ---- compute stream ----
        # head: 256-col slivers of Q0/K0 start the exp stream ~3us earlier
        project(q0a, wq_sb, qpt_sb, 0, "pq0a", lo=0, w=256,
                copy_eng="scalar")
        project(k0a, wk_sb, kpt_sb, 0, "pk0a", lo=0, w=256)
        z_head_part(0, 0)
        project(q0b, wq_sb, qpt_sb, 0, "pq0b", lo=256, w=256, t_lo=256,
                copy_eng="scalar")
        z_head_part(0, 1)
        project(k0b, wk_sb, kpt_sb, 0, "pk0b", lo=256, w=256, t_lo=256)
        z_exp(0, 1)
        project(tq[1], wq_sb, qpt_sb, 1, "pq1")
        z_exp(1, 0)
        z_exp(1, 1)
        project(tk[1], wk_sb, kpt_sb, 1, "pk1")
        z_exp(0, 2)
        z_exp(0, 3)
        z_exp(1, 2)
        z_exp(1, 3)
        project_v(tv[0], 0)
        # c=2
        project(tk[2], wk_sb, kpt_sb, 2, "pk2")
        z_exp(0, 4)
        z_exp(0, 5)
        z_exp(1, 4)
        z_exp(1, 5)
        project(tq[2], wq_sb, qpt_sb, 2, "pq2")
        z_exp(2, 0)
        z_exp(2, 1)
        z_exp(2, 2)
        project_v(tv[1], 1)
        # K3-dependent last exps of groups 0-2, pulled as early as K3 allows
        project(tk[3], wk_sb, kpt_sb, 3, "pk3")
        z_exp(0, 6)
        z_exp(0, 7)
        z_exp(1, 6)
        z_exp(1, 7)
        project_v(tv[2], 2)
        z_exp(2, 6)
        z_exp(2, 7)
        z_exp(2, 3)
        z_exp(2, 4)
        z_exp(2, 5)
        project_v(tv[3], 3)
        h00 = unit_front(0, 0)
        h01 = unit_front(0, 1)
        unit_back(0, 0, h00, pe_tr=True)
        h02 = unit_front(0, 2)
        unit_back(0, 1, h01, pe_tr=True)
        h03 = unit_front(0, 3)
        unit_back(0, 2, h02, pe_tr=True)
        project(tq[3], wq_sb, qpt_sb, 3, "pq3")
        unit_back(0, 3, h03)
        z_pack(3, 0)
        fb = []
        fb.append((1, 0, unit_front(1, 0)))
        z_pack(3, 1)
        fb.append((1, 1, unit_front(1, 1)))
        unit_back(*fb.pop(0))
        z_pack(4, 0)
        fb.append((1, 2, unit_front(1, 2)))
        unit_back(*fb.pop(0))
        z_pack(4, 1)
        fb.append((3, 0, unit_front(3, 0)))   # gate: zp3 exps, just landed
        unit_back(*fb.pop(0))
        z_pack(5, 0)
        fb.append((1, 3, unit_front(1, 3)))
        unit_back(*fb.pop(0))
        z_pack(5, 1)
        fb.append((2, 0, unit_front(2, 0)))
        unit_back(*fb.pop(0))
        z_pack(6, 0)
        z_pack(6, 1)
        fb.append((4, 0, unit_front(4, 0)))   # gate: zp4 exps
        unit_back(*fb.pop(0))
        fb.append((2, 1, unit_front(2, 1)))
        unit_back(*fb.pop(0))
        fb.append((2, 2, unit_front(2, 2)))
        unit_back(*fb.pop(0))
        fb.append((5, 0, unit_front(5, 0)))   # gate: zp5 exps
        unit_back(*fb.pop(0), tail=True)
        fb.append((2, 3, unit_front(2, 3)))
        unit_back(*fb.pop(0), tail=True)
        unit_back(*fb.pop(0), tail=True)

        # group 6: the only chain after the last exp
        hsq6 = unit_front(6, 0, tail=True)
        unit_back(6, 0, hsq6, tail=True)


def _build_nc():
    nc = bacc.Bacc("TRN2", target_bir_lowering=False, debug=False,
                   num_devices=NCORES)
    aps = {
        "qt": nc.dram_tensor("qt", [D_X, S], F16, kind="ExternalInput").ap(),
        "kt": nc.dram_tensor("kt", [D_X, S], F16, kind="ExternalInput").ap(),
        "vt": nc.dram_tensor("vt", [D_X, S], F16, kind="ExternalInput").ap(),
        "wq": nc.dram_tensor("wq", [128, D_X], F16, kind="ExternalInput").ap(),
        "wk": nc.dram_tensor("wk", [128, D_X], F16, kind="ExternalInput").ap(),
        "wv": nc.dram_tensor("wv", [128, D_X], F16, kind="ExternalInput").ap(),
        "wot": nc.dram_tensor("wot", [HD, D_M], F16, kind="ExternalInput").ap(),
        "ident": nc.dram_tensor("ident", [128, 128], F16, kind="ExternalInput").ap(),
        "out": nc.dram_tensor("out", [S, D_M], F16, kind="ExternalOutput").ap(),
    }
    with tile.TileContext(nc) as tc:
        with nc.allow_low_precision(reason="fp16 matmul/softmax pipeline"):
            _emit(tc, nc, aps)
    nc.compile()
    return nc


def kernel(**inputs):
    global LAST_EXEC_NS, _NC_CACHE
    Q = np.asarray(inputs["Q"], dtype=np.float32)
    K = np.asarray(inputs["K"], dtype=np.float32)
    V = np.asarray(inputs["V"], dtype=np.float32)
    W_q = np.asarray(inputs["W_q"], dtype=np.float32)
    W_k = np.asarray(inputs["W_k"], dtype=np.float32)
    W_v = np.asarray(inputs["W_v"], dtype=np.float32)
    W_o = np.asarray(inputs["W_o"], dtype=np.float32)

    def _pack_w(W, h0):
        # device SBUF layout [p, c*128+f] = W[c*128+p, f]; full-rate DMA rows
        w = np.concatenate([W[h0 + i] for i in range(HPC)], axis=1)  # (D_X, HD)
        return np.ascontiguousarray(
            w.reshape(NXC, 128, HD).transpose(1, 0, 2).reshape(128, NXC * HD)
            .astype(np.float16))

    QT = np.ascontiguousarray(Q.T.astype(np.float16))
    KT = np.ascontiguousarray(K.T.astype(np.float16))
    VT = np.ascontiguousarray(V.T.astype(np.float16))
    ident = np.eye(128, dtype=np.float16)
    in_maps = []
    for c in range(NCORES):
        h0 = HPC * c
        in_maps.append({
            "qt": QT, "kt": KT, "vt": VT,
            "wq": _pack_w(W_q, h0), "wk": _pack_w(W_k, h0),
            "wv": _pack_w(W_v, h0),
            "wot": np.ascontiguousarray(
                W_o[:, c * HD:(c + 1) * HD].T.astype(np.float16)),
            "ident": ident,
        })

    if _NC_CACHE is None:
        _NC_CACHE = _build_nc()
    nc = _NC_CACHE

    trace = bool(os.environ.get("MHA_TRACE"))
    res = None
    if trace:
        try:
            res = run_bass_kernel_spmd(nc, in_maps, list(range(NCORES)),
                                       trace=True)
        except Exception as e:  # profiling infra unavailable -> run untraced
            print(f"[kernel] traced run failed ({e!r}); falling back")
            res = None
    if res is None:
        res = run_bass_kernel_spmd(nc, in_maps, list(range(NCORES)))

    LAST_EXEC_NS = getattr(res, "exec_time_ns", None)

    out = np.zeros((S, D_M), np.float32)
    for r in res.results:
        out += r["out"].astype(np.float32)
    return out


# revision 6
# speedup vs baseline: 1.1407x; 1.1407x over previous
"""Multi-head attention (16 heads, S=2048, d_model=1024, d_head=64) on 8 TRN2
NeuronCores, tensor-parallel over heads (2 heads per core).

v3, restructured around the TimelineSim cost model (matmul cost = output
free-size rows; ACT/DVE/Pool cost = max-operand free size x engine cycle):

  * AV matmuls transposed: out[sq=128, dv+1=65], exp tile stationary -> 65
    rows per accumulation step; softmax normalize is a per-partition
    tensor_scalar against the ride-along denominator column.
  * everything 2-byte through the matmuls; exp computes exp(z/8 - 4) so
    scores fit fp16, bias cancels in the normalize.
  * engine rebalance: 16 of 64 exp tiles run as a Schraudolph bit-trick
    split across DVE (tmp = z*A+B, the only PSUM-capable stage) and Pool
    (int16 max+convert, SBUF-only) -- Pool is otherwise idle and ACT drops
    from ~72us busy to ~55us; copies split ~28 DVE / ~4 ACT.
  * two HWDGE queues: SP carries wk + K/V chunks + out DMAs, ACT carries
    wq/wv/wot + Q chunks + hsq DMA-transposes. First Q/K chunks are 256
    cols and the first z pair-tile exps split by sq-half, pulling the
    first exp from ~9.4us to ~6.5us.
  * hsq [sq,hd] -> headst [hd,sq] via DMA-engine XBAR transpose (14ns per
    16x128 tile) instead of PE transpose for all but the first 3 units
    (those fire while input DMA still owns the device) -- saves 2k PE rows
    and the tr PSUM tile.

Sq splits into groups [512,512,512,128,128,128,128]; packed 128-wide tail
groups keep exps dense and make each finish chain a single sq-tile.
PSUM: 2x[128,1024] z + 2x[128,130] paired-head AV accumulators +
2x[128,512] proj/outproj = exactly 8 banks.
"""

import os

import numpy as np

import concourse.bass as bass
import concourse.tile as tile
from concourse import bacc, mybir
from concourse.bass_utils import run_bass_kernel_spmd

HEADS, D_K, D_V, D_X, D_M, S = 16, 64, 64, 1024, 1024, 2048
NCORES = 8
HPC = HEADS // NCORES          # heads per core
HD = HPC * D_K                 # 128: stacked head dim per core
SQW = 512                      # sq group width
NSQ = S // SQW                 # 4 groups
SKW = 128                      # sk chunk width (partition dim)
NSK = S // SKW                 # 16
NXC = D_X // 128               # 8 contraction chunks for projections
NJJ = NSK // 2                 # 8 skc-pairs (one [128,1024] z tile each)
GROUPS = [(0, 512), (512, 512), (1024, 512),
          (1536, 128), (1664, 128), (1792, 128), (1920, 128)]

F32 = mybir.dt.float32
F16 = mybir.dt.float16
I16 = mybir.dt.int16
EXP = mybir.ActivationFunctionType.Exp
# Schraudolph fp16 exp constants: exp(z/8-4) = 2^(z*0.125*log2e - 4*log2e),
# int16(1024*(t + 15 - C)) bitcast as fp16 ~ 2^t; C tuned for min rms (1.9%)
LOG2E = 1.4426950408889634
SCH_A = 0.125 * LOG2E * 1024.0
SCH_B = 1024.0 * (15.0 - 4.0 * LOG2E - 0.0545)

# z tiles whose exp runs on DVE (stage1) + Pool (stage2) instead of ACT
SPLIT_TILES = {(0, 3), (0, 5), (1, 1), (1, 3), (1, 5), (2, 1), (2, 3), (2, 5)}

LAST_EXEC_NS = None
_NC_CACHE = None


def _emit(tc, nc, aps):
    from contextlib import ExitStack

    qt, kt, vt, wq, wk, wv, wot, ident, out = (
        aps["qt"], aps["kt"], aps["vt"], aps["wq"], aps["wk"], aps["wv"],
        aps["wot"], aps["ident"], aps["out"],
    )

    with ExitStack() as ctx:
        wpool = ctx.enter_context(tc.tile_pool(name="weights", bufs=1))
        proj = ctx.enter_context(tc.tile_pool(name="proj", bufs=1))
        inp = ctx.enter_context(tc.tile_pool(name="inp", bufs=5))
        etp = ctx.enter_context(tc.tile_pool(name="et", bufs=46))
        hsqp = ctx.enter_context(tc.tile_pool(name="hsq", bufs=10))
        outp = ctx.enter_context(tc.tile_pool(name="outs", bufs=6))
        smalls = ctx.enter_context(tc.tile_pool(name="smalls", bufs=4))
        ps_z = ctx.enter_context(tc.tile_pool(name="ps_z", bufs=2, space="PSUM"))
        ps_av = ctx.enter_context(tc.tile_pool(name="ps_av", bufs=2, space="PSUM"))
        ps_pr = ctx.enter_context(tc.tile_pool(name="ps_pr", bufs=2, space="PSUM"))

        # ---- persistent SBUF tensors ----
        wq_sb = wpool.tile([128, D_X], F16, tag="wq")     # (xc p) stacked chunks
        wk_sb = wpool.tile([128, D_X], F16, tag="wk")
        wv_sb = wpool.tile([128, D_X], F16, tag="wv")
        wot_sb = wpool.tile([HD, D_M], F16, tag="wot")
        ident_sb = wpool.tile([128, 128], F16, tag="ident")
        qpt_sb = proj.tile([HD, S], F16, tag="qpt")
        kpt_sb = proj.tile([HD, S], F16, tag="kpt")
        # VpAug: per (h, skc) a (128 sk, 65) block: cols 0-63 = Vp, col 64 = 1
        vpa_sb = proj.tile([128, HPC * NSK * 65], F16, tag="vpa")
        headst_sb = proj.tile([HD, S], F16, tag="headst")

        def load_w(w_dram, w_sb, q=None):
            (q or nc.sync).dma_start(w_sb[:], w_dram)

        def load_chunk(tt_dram, c, name, lo=0, w=SQW, tag="inp", q=None):
            """One DMA: all 8 xc strips of cols [c*512+lo, +w) -> (128, 8, w)."""
            t = inp.tile([128, NXC, w], F16, tag=tag, name=name,
                         bufs=4 if tag == "inp0" else None)
            (q or nc.sync).dma_start(
                t[:],
                tt_dram.rearrange("(xc p) s -> p xc s", p=128)[
                    :, :, c * SQW + lo:c * SQW + lo + w
                ],
            )
            return t

        def project(t, w_sb, dst_sb, c, name, lo=0, w=SQW, copy_eng=None,
                    t_lo=0):
            """dst_sb[:, c*512+lo : +w] = W.T @ X.T chunk cols (fp16)."""
            ps = ps_pr.tile([128, w], F32, tag="pr", name=name,
                            padded_shape=[128, SQW])
            for xc in range(NXC):
                nc.tensor.matmul(
                    ps[:],
                    w_sb[:, xc * 128:(xc + 1) * 128],
                    t[:, xc, lo - t_lo:lo - t_lo + w],
                    start=(xc == 0),
                    stop=(xc == NXC - 1),
                )
            dst = dst_sb[:, c * SQW + lo:c * SQW + lo + w]
            if copy_eng == "split":
                nc.vector.tensor_copy(dst[:, 0:w // 2], ps[:, 0:w // 2])
                nc.scalar.copy(dst[:, w // 2:w], ps[:, w // 2:w])
            elif copy_eng == "scalar":
                nc.scalar.copy(dst, ps[:])
            else:
                nc.vector.tensor_copy(dst, ps[:])

        def project_v(t, c):
            """VpAug sk-chunks for 512-chunk c: Vp = VT_chunk.T @ Wv directly
            in (sk, hd) layout."""
            for j in range(SQW // SKW):
                skc = c * (SQW // SKW) + j
                ps = ps_pr.tile([128, HD], F32, tag="pr", name=f"vp_{skc}",
                                padded_shape=[128, SQW])
                for xc in range(NXC):
                    nc.tensor.matmul(
                        ps[:],
                        t[:, xc, j * SKW:(j + 1) * SKW],
                        wv_sb[:, xc * 128:(xc + 1) * 128],
                        start=(xc == 0),
                        stop=(xc == NXC - 1),
                    )
                # both heads in one strided copy: vpa block stride is 16*65
                nc.vector.tensor_copy(
                    vpa_sb[:].rearrange("p (h f) -> p h f", h=HPC)[
                        :, :, skc * 65:skc * 65 + 64],
                    ps[:].rearrange("p (h f) -> p h f", h=HPC),
                )

        ets = {}  # (h, g, jj) -> ET tile awaiting its AV matmuls
        zhead = {}  # (h, jj) -> (z_ps, et) for the sq-split head tiles

        def z_head_part(jj, part):
            """First z pair-tiles, computed in sq-256 halves so the exp
            stream starts as soon as the first 256 cols of Q land."""
            lo = part * 256
            for h in range(HPC):
                if part == 0:
                    z_ps = ps_z.tile([128, 2 * SQW], F32, tag="z",
                                     name=f"z_{h}_0_{jj}")
                    et = etp.tile([128, 2 * SQW], F16, tag="et",
                                  name=f"et_{h}_0_{jj}")
                    zhead[(h, jj)] = (z_ps, et)
                    ets[(h, 0, jj)] = et
                z_ps, et = zhead[(h, jj)]
                for half in range(2):
                    skc = 2 * jj + half
                    nc.tensor.matmul(
                        z_ps[:, half * SQW + lo:half * SQW + lo + 256],
                        kpt_sb[h * 64:(h + 1) * 64, skc * SKW:(skc + 1) * SKW],
                        qpt_sb[h * 64:(h + 1) * 64, lo:lo + 256],
                        start=True,
                        stop=True,
                    )
                zin = z_ps[:].rearrange("p (c w) -> p c w", c=2)[
                    :, :, lo:lo + 256]
                eout = et[:].rearrange("p (c w) -> p c w", c=2)[
                    :, :, lo:lo + 256]
                nc.scalar.activation(eout, zin, EXP, scale=1.0 / 8.0,
                                     bias=bias_sb[:])

        def z_exp(g, jj):
            """Per head: one [128, 2W] z tile (skc pair 2jj,2jj+1) + exp.

            exp(z/8 - 4): the -4 bias keeps the scores in fp16 range and
            cancels in the normalize. Tiles in SPLIT_TILES compute a
            Schraudolph bit-trick exp instead: DVE does tmp = z*A+B (the
            PSUM read), Pool does the int16 max/convert; ~1.9% rms score
            error that largely cancels num/denom.
            """
            lo, w = GROUPS[g]
            split = (g, jj) in SPLIT_TILES
            for h in range(HPC):
                z_ps = ps_z.tile([128, SQW + w], F32, tag="z",
                                 name=f"z_{h}_{g}_{jj}",
                                 padded_shape=[128, 2 * SQW])
                for half in range(2):
                    skc = 2 * jj + half
                    nc.tensor.matmul(
                        z_ps[:, half * SQW:half * SQW + w],
                        kpt_sb[h * 64:(h + 1) * 64, skc * SKW:(skc + 1) * SKW],
                        qpt_sb[h * 64:(h + 1) * 64, lo:lo + w],
                        start=True,
                        stop=True,
                    )
                et = etp.tile([128, SQW + w], F16, tag="et",
                              name=f"et_{h}_{g}_{jj}",
                              padded_shape=[128, 2 * SQW])
                if split:
                    tmp = smalls.tile([128, SQW + w], F32, tag="tmp", bufs=3,
                                      name=f"tmp_{h}_{g}_{jj}",
                                      padded_shape=[128, 2 * SQW])
                    nc.vector.tensor_scalar(
                        tmp[:], z_ps[:], SCH_A, SCH_B,
                        mybir.AluOpType.mult, mybir.AluOpType.add)
                    nc.gpsimd.tensor_scalar_max(
                        et[:].bitcast(I16), tmp[:], 0.0)
                else:
                    nc.scalar.activation(et[:], z_ps[:], EXP,
                                         scale=1.0 / 8.0, bias=bias_sb[:])
                ets[(h, g, jj)] = et

        def z_pack(g, i):
            """128-wide group: one tile packs 8 skc at col k*128."""
            lo, w = GROUPS[g]
            for h in range(HPC):
                z_ps = ps_z.tile([128, 2 * SQW], F32, tag="z",
                                 name=f"zp_{g}_{h}_{i}")
                for k in range(8):
                    skc = 8 * i + k
                    nc.tensor.matmul(
                        z_ps[:, k * 128:(k + 1) * 128],
                        kpt_sb[h * 64:(h + 1) * 64, skc * SKW:(skc + 1) * SKW],
                        qpt_sb[h * 64:(h + 1) * 64, lo:lo + w],
                        start=True,
                        stop=True,
                    )
                et = etp.tile([128, 2 * SQW], F16, tag="et",
                              name=f"etp_{g}_{h}_{i}")
                nc.scalar.activation(et[:], z_ps[:], EXP,
                                     scale=1.0 / 8.0, bias=bias_sb[:])
                ets[(h, g, i)] = et

        # PSUM accumulation groups must be contiguous per bank, so AV runs
        # as per-(h, sq-128-subtile) bursts of 16 back-to-back matmuls.
        _avalt = [0]

        def alloc_acc(name):
            _avalt[0] ^= 1
            return ps_av.tile([128, 130], F32, tag=f"av{_avalt[0]}", bufs=1,
                              name=name)

        def av_run(g, h, m, acc0):
            acc = acc0[:, h * 65:(h + 1) * 65]
            if GROUPS[g][1] == 128:
                for skc in range(NSK):
                    et = ets[(h, g, skc // 8)]
                    vb = (h * NSK + skc) * 65
                    nc.tensor.matmul(
                        acc[:],
                        et[:, (skc % 8) * 128:(skc % 8 + 1) * 128],
                        vpa_sb[:, vb:vb + 65],
                        start=(skc == 0),
                        stop=(skc == NSK - 1),
                    )
                return
            for jj in range(NJJ):
                et = ets[(h, g, jj)]
                for half in range(2):
                    skc = 2 * jj + half
                    vb = (h * NSK + skc) * 65
                    nc.tensor.matmul(
                        acc[:],
                        et[:, half * SQW + m * 128:half * SQW + (m + 1) * 128],
                        vpa_sb[:, vb:vb + 65],
                        start=(skc == 0),
                        stop=(skc == NSK - 1),
                    )

        def unit_front(g, m, tail=False):
            """AV runs + normalize -> hsq in SBUF."""
            t = GROUPS[g][0] // 128 + m
            acc = alloc_acc(f"acc_{t}")
            for h in range(HPC):
                av_run(g, h, m, acc)
            hsq = hsqp.tile([128, HD], F16, tag="hsq", name=f"hsq_{t}")
            rec = smalls.tile([128, 2], F32, tag="rec", name=f"rec_{t}")
            nc.vector.reciprocal(
                rec[:].rearrange("p (h c) -> p h c", c=1),
                acc[:].rearrange("p (h c) -> p h c", c=65)[:, :, 64:65])
            for h in range(HPC):
                dsth = hsq[:, h * 64:(h + 1) * 64]
                if tail and h == 1:
                    nc.scalar.mul(dsth, acc[:, h * 65:h * 65 + 64],
                                  rec[:, h:h + 1])
                else:
                    nc.vector.tensor_scalar_mul(
                        dsth, acc[:, h * 65:h * 65 + 64], rec[:, h:h + 1])
            return hsq

        def unit_back(g, m, hsq, tail=False):
            """transpose + outproj + DMA; a z-slot after its front."""
            t = GROUPS[g][0] // 128 + m
            hdst = headst_sb[:, t * 128:(t + 1) * 128]
            tr = ps_pr.tile([128, SQW], F16, tag="pr", name=f"tr_{t}")
            nc.tensor.transpose(tr[:, 0:128], hsq[:], ident_sb[:])
            if tail:
                nc.scalar.copy(hdst, tr[:, 0:128])
            else:
                nc.vector.tensor_copy(hdst, tr[:, 0:128])
            ot = outp.tile([128, D_M], F16, tag="ot", name=f"ot_{t}")
            for dmc in range(D_M // SQW):
                op = ps_pr.tile([128, SQW], F32, tag="pr", name=f"op_{t}_{dmc}")
                nc.tensor.matmul(
                    op[:],
                    headst_sb[:, t * 128:(t + 1) * 128],
                    wot_sb[:, dmc * SQW:(dmc + 1) * SQW],
                    start=True,
                    stop=True,
                )
                dst = ot[:, dmc * SQW:(dmc + 1) * SQW]
                if (tail and dmc % 2) or (not tail and (2 * t + dmc) % 8 == 3):
                    nc.scalar.copy(dst, op[:])
                else:
                    nc.vector.tensor_copy(dst, op[:])
                if tail:
                    nc.sync.dma_start(
                        out[t * 128:(t + 1) * 128, dmc * SQW:(dmc + 1) * SQW],
                        ot[:, dmc * SQW:(dmc + 1) * SQW])
            if not tail:
                nc.sync.dma_start(out[t * 128:(t + 1) * 128, :], ot[:])

        # ---- gpsimd constants (no DMA needed) ----
        nc.gpsimd.memset(
            vpa_sb[:].rearrange("p (c f) -> p c f", f=65)[:, :, 64:65], 1.0)
        bias_sb = wpool.tile([128, 1], F32, tag="bias")
        nc.gpsimd.memset(bias_sb[:], -4.0)

        # absorb the 1.3us exp table load inside the initial DMA window;
        # emitted before the ACT-queue DMAs so it dispatches immediately
        warm = smalls.tile([128, 1], F32, tag="warm")
        nc.scalar.activation(warm[:], bias_sb[:], EXP, scale=1.0)

        # ---- DMA streams ----
        # SP queue: wk + K/V chunks (+ out DMAs later)
        # ACT queue: wq/wv/wot + Q chunks (+ hsq transposes later)
        tk, tq, tv = {}, {}, {}
        load_w(wk, wk_sb, q=nc.sync)
        load_w(wq, wq_sb, q=nc.scalar)
        q0a = load_chunk(qt, 0, "qc_0a", lo=0, w=256, tag="inp0", q=nc.scalar)
        k0a = load_chunk(kt, 0, "kc_0a", lo=0, w=256, tag="inp0", q=nc.sync)
        q0b = load_chunk(qt, 0, "qc_0b", lo=256, w=256, tag="inp0",
                         q=nc.scalar)
        k0b = load_chunk(kt, 0, "kc_0b", lo=256, w=256, tag="inp0", q=nc.sync)
        tq[1] = load_chunk(qt, 1, "qc_1", q=nc.scalar)
        tk[1] = load_chunk(kt, 1, "kc_1", q=nc.sync)
        nc.sync.dma_start(ident_sb[:], ident)
        load_w(wv, wv_sb, q=nc.scalar)
        tv[0] = load_chunk(vt, 0, "vc_0", q=nc.sync)
        tq[2] = load_chunk(qt, 2, "qc_2", q=nc.scalar)
        tk[2] = load_chunk(kt, 2, "kc_2", q=nc.sync)
        tv[1] = load_chunk(vt, 1, "vc_1", q=nc.sync)
        load_w(wot, wot_sb, q=nc.scalar)
        tk[3] = load_chunk(kt, 3, "kc_3", q=nc.sync)
        tq[3] = load_chunk(qt, 3, "qc_3", q=nc.scalar)
        tv[2] = load_chunk(vt, 2, "vc_2", q=nc.sync)
        tv[3] = load_chunk(vt, 3, "vc_3", q=nc.sync)

        # burn the PE pstate ramp on a few junk matmuls over memset data
        jsrc = wpool.tile([128, 128], F16, tag="jsrc")
        nc.gpsimd.memset(jsrc[:], 0.0)
        junk = ps_pr.tile([128, SQW], F32, tag="pr", name="junk")
        for _ in range(20):
            nc.tensor.matmul(junk[:, 0:128], jsrc[:], jsrc[:],
                             start=True, stop=True)

        # ---- compute stream ----
        # head: 256-col slivers of Q0/K0 start the exp stream ~3us earlier
        project(q0a, wq_sb, qpt_sb, 0, "pq0a", lo=0, w=256,
                copy_eng="scalar")
        project(k0a, wk_sb, kpt_sb, 0, "pk0a", lo=0, w=256)
        z_head_part(0, 0)
        project(q0b, wq_sb, qpt_sb, 0, "pq0b", lo=256, w=256, t_lo=256,
                copy_eng="scalar")
        z_head_part(0, 1)
        project(k0b, wk_sb, kpt_sb, 0, "pk0b", lo=256, w=256, t_lo=256)
        z_exp(0, 1)
        project(tq[1], wq_sb, qpt_sb, 1, "pq1")
        z_exp(1, 0)
        z_exp(1, 1)
        project(tk[1], wk_sb, kpt_sb, 1, "pk1")
        z_exp(0, 2)
        z_exp(0, 3)
        z_exp(1, 2)
        z_exp(1, 3)
        project_v(tv[0], 0)
        # c=2
        project(tk[2], wk_sb, kpt_sb, 2, "pk2")
        z_exp(0, 4)
        z_exp(0, 5)
        z_exp(1, 4)
        z_exp(1, 5)
        project(tq[2], wq_sb, qpt_sb, 2, "pq2")
        z_exp(2, 0)
        z_exp(2, 1)
        z_exp(2, 2)
        project_v(tv[1], 1)
        # K3-dependent last exps of groups 0-2, pulled as early as K3 allows
        project(tk[3], wk_sb, kpt_sb, 3, "pk3")
        z_exp(0, 6)
        z_exp(0, 7)
        z_exp(1, 6)
        z_exp(1, 7)
        project_v(tv[2], 2)
        z_exp(2, 6)
        z_exp(2, 7)
        z_exp(2, 3)
        z_exp(2, 4)
        z_exp(2, 5)
        project_v(tv[3], 3)
        h00 = unit_front(0, 0)
        h01 = unit_front(0, 1)
        unit_back(0, 0, h00)
        h02 = unit_front(0, 2)
        unit_back(0, 1, h01)
        h03 = unit_front(0, 3)
        unit_back(0, 2, h02)
        project(tq[3], wq_sb, qpt_sb, 3, "pq3")
        unit_back(0, 3, h03)
        z_pack(3, 0)
        fb = []
        fb.append((1, 0, unit_front(1, 0)))
        z_pack(3, 1)
        fb.append((1, 1, unit_front(1, 1)))
        unit_back(*fb.pop(0))
        z_pack(4, 0)
        fb.append((1, 2, unit_front(1, 2)))
        unit_back(*fb.pop(0))
        z_pack(4, 1)
        fb.append((3, 0, unit_front(3, 0)))   # gate: zp3 exps, just landed
        unit_back(*fb.pop(0))
        z_pack(5, 0)
        fb.append((1, 3, unit_front(1, 3)))
        unit_back(*fb.pop(0))
        z_pack(5, 1)
        fb.append((2, 0, unit_front(2, 0)))
        unit_back(*fb.pop(0))
        z_pack(6, 0)
        z_pack(6, 1)
        fb.append((4, 0, unit_front(4, 0)))   # gate: zp4 exps
        unit_back(*fb.pop(0))
        fb.append((2, 1, unit_front(2, 1)))
        unit_back(*fb.pop(0))
        fb.append((2, 2, unit_front(2, 2)))
        unit_back(*fb.pop(0))
        fb.append((5, 0, unit_front(5, 0)))   # gate: zp5 exps
        unit_back(*fb.pop(0), tail=True)
        fb.append((2, 3, unit_front(2, 3)))
        unit_back(*fb.pop(0), tail=True)
        unit_back(*fb.pop(0), tail=True)

        # group 6: the only chain after the last exp
        hsq6 = unit_front(6, 0, tail=True)
        unit_back(6, 0, hsq6, tail=True)


def _build_nc():
    nc = bacc.Bacc("TRN2", target_bir_lowering=False, debug=False,
                   num_devices=NCORES)
    aps = {
        "qt": nc.dram_tensor("qt", [D_X, S], F16, kind="ExternalInput").ap(),
        "kt": nc.dram_tensor("kt", [D_X, S], F16, kind="ExternalInput").ap(),
        "vt": nc.dram_tensor("vt", [D_X, S], F16, kind="ExternalInput").ap(),
        "wq": nc.dram_tensor("wq", [128, D_X], F16, kind="ExternalInput").ap(),
        "wk": nc.dram_tensor("wk", [128, D_X], F16, kind="ExternalInput").ap(),
        "wv": nc.dram_tensor("wv", [128, D_X], F16, kind="ExternalInput").ap(),
        "wot": nc.dram_tensor("wot", [HD, D_M], F16, kind="ExternalInput").ap(),
        "ident": nc.dram_tensor("ident", [128, 128], F16, kind="ExternalInput").ap(),
        "out": nc.dram_tensor("out", [S, D_M], F16, kind="ExternalOutput").ap(),
    }
    with tile.TileContext(nc) as tc:
        with nc.allow_low_precision(reason="fp16 matmul/softmax pipeline"):
            _emit(tc, nc, aps)
    nc.compile()
    return nc


def kernel(**inputs):
    global LAST_EXEC_NS, _NC_CACHE
    Q = np.asarray(inputs["Q"], dtype=np.float32)
    K = np.asarray(inputs["K"], dtype=np.float32)
    V = np.asarray(inputs["V"], dtype=np.float32)
    W_q = np.asarray(inputs["W_q"], dtype=np.float32)
    W_k = np.asarray(inputs["W_k"], dtype=np.float32)
    W_v = np.asarray(inputs["W_v"], dtype=np.float32)
    W_o = np.asarray(inputs["W_o"], dtype=np.float32)

    def _pack_w(W, h0):
        # device SBUF layout [p, c*128+f] = W[c*128+p, f]; full-rate DMA rows
        w = np.concatenate([W[h0 + i] for i in range(HPC)], axis=1)  # (D_X, HD)
        return np.ascontiguousarray(
            w.reshape(NXC, 128, HD).transpose(1, 0, 2).reshape(128, NXC * HD)
            .astype(np.float16))

    QT = np.ascontiguousarray(Q.T.astype(np.float16))
    KT = np.ascontiguousarray(K.T.astype(np.float16))
    VT = np.ascontiguousarray(V.T.astype(np.float16))
    ident = np.eye(128, dtype=np.float16)
    in_maps = []
    for c in range(NCORES):
        h0 = HPC * c
        in_maps.append({
            "qt": QT, "kt": KT, "vt": VT,
            "wq": _pack_w(W_q, h0), "wk": _pack_w(W_k, h0),
            "wv": _pack_w(W_v, h0),
            "wot": np.ascontiguousarray(
                W_o[:, c * HD:(c + 1) * HD].T.astype(np.float16)),
            "ident": ident,
        })

    if _NC_CACHE is None:
        _NC_CACHE = _build_nc()
    nc = _NC_CACHE

    trace = bool(os.environ.get("MHA_TRACE"))
    res = None
    if trace:
        try:
            res = run_bass_kernel_spmd(nc, in_maps, list(range(NCORES)),
                                       trace=True)
        except Exception as e:  # profiling infra unavailable -> run untraced
            print(f"[kernel] traced run failed ({e!r}); falling back")
            res = None
    if res is None:
        res = run_bass_kernel_spmd(nc, in_maps, list(range(NCORES)))

    LAST_EXEC_NS = getattr(res, "exec_time_ns", None)

    out = np.zeros((S, D_M), np.float32)
    for r in res.results:
        out += r["out"].astype(np.float32)
    return out
